# revision 69
# baseline (speedup 1.0000x reference)
"""Trainium2 Bass kernel for nn_OmegaEntangle (E^T C E with entangle coefficients).

Math (validated vs reference):
  p_i = sum_j v_ij^2 ; m_i = mean_j v_ij
  C[i,j] = mask(i<j) * sqrt(p_i p_j) * (m_i + 1j*m_j) / sqrt(m_i^2 + m_j^2)
  out = E^T C E   (complex, E real)  ->  out_re = E^T Cr E, out_im = E^T Ci E

Sharding: data-parallel over the 2048 OUTPUT COLUMNS (256 per core), with the
p/m reduction row-sharded (64 rows per core).

Two NEFF launches (a device collective costs far more than host concat of the
tiny reduction result on this platform):
  Kernel A: each core reduces its [64, 32768] vuln shard into per-tile
    partial sums [128, 2*NT]; the host finishes the (tiny) reduction.
  Kernel B: each core broadcasts p/m across partitions via a K=1 matmul,
    derives the coefficient vectors, builds C^T in bf16, computes
    T = C @ E[:, cols] and out[:, cols] = E^T @ T in bf16 matmuls
    (fp32 PSUM), writes bf16 [256, 2048] slabs.
Host concatenates slabs along columns -> [2048, 2048] complex64.

Perf notes:
  - TRN2 has two HWDGE queues (SP + Activation), ~190 GB/s each; every
    large DMA set is split across both to reach the ~358 GB/s core limit.
  - bf16 matmul streams 1 row/cycle (same as f32r) at half the DMA bytes.
  - PE p-state ramps to full clock only after ~3us of continuous busy;
    warmup matmuls keep it streaming until the C tiles are ready.
"""

import numpy as np
import ml_dtypes

import concourse.bass as bass
import concourse.mybir as mybir
import concourse.tile as tile
from concourse import bacc
from concourse.bass_utils import run_bass_kernel_spmd

D = 512          # number of domains
V = 32768        # vuln dim
S = 2048         # sup (embed) dim
NCORES = 8
ROWS_PER_CORE = D // NCORES          # 64
COLS_PER_CORE = S // NCORES          # 256
KT = D // 128                         # 4 contraction tiles
INV_V = 1.0 / V

# kernel A tiling: narrower tiles at the end shorten the compute tail
WIDTHS_A = [2048] * 6 + [1024] * 4
NT = len(WIDTHS_A)

WARMUP = 6                            # PE warm matmuls until C tiles are ready
POST_FILL = 6                         # fillers covering the T->out copy gap

F32 = mybir.dt.float32
F32R = mybir.dt.float32r
BF16 = mybir.dt.bfloat16
AF = mybir.ActivationFunctionType
ALU = mybir.AluOpType

_CACHE = {}


def _tf32_round(x):
    xi = np.ascontiguousarray(x, dtype=np.float32).view(np.uint32)
    return ((xi + np.uint32(0x1000)) & np.uint32(0xFFFFE000)).view(np.float32)


def build_kernel_a():
    """Reduce kernel: per-tile partial p/msum over the 64-row vuln shard.

    DMA-bound: 8 MiB/core of vulns split across both HWDGE queues.
    Scalar does Square+accum, vector does row-sum reduce; host finishes.
    """
    nc = bacc.Bacc("TRN2", target_bir_lowering=False, debug=False, num_devices=NCORES)

    v128 = nc.dram_tensor("v128", [128, V * ROWS_PER_CORE // 128], F32,
                          kind="ExternalInput")
    out_pm = nc.dram_tensor("out_pm", [128, 2 * NT], F32, kind="ExternalOutput")

    with tile.TileContext(nc) as tc:
        with (
            tc.tile_pool(name="vin", bufs=5) as vin_pool,
            tc.tile_pool(name="scr", bufs=2) as scr_pool,
            tc.tile_pool(name="small", bufs=1) as small_pool,
        ):
            # all DMA triggers first, spread over 3 queues (2 HWDGE + SWDGE).
            # scalar goes last in the rotation: its ACT_TABLE_LOAD (for the
            # Square activation) delays its first trigger by ~1.3us.
            vts = []
            off = 0
            # arrival order must match program order: sync starts ~8.7us,
            # scalar ~10 (after its ACT table), SWDGE ~12 — so t0/t1/t2 land
            # in sequence and the per-tile compute pipeline never stalls
            engs = [nc.sync, nc.scalar, nc.gpsimd]
            for t, w in enumerate(WIDTHS_A):
                vt = vin_pool.tile([128, 2048], F32, name=f"vt{t}", tag="vt")
                engs[t % 3].dma_start(vt[:, 0:w], v128[:, off : off + w])
                off += w
                vts.append(vt)

            # squares on scalar, row-sums on vector (each ~18-20us of engine
            # time; one engine cannot absorb both)
            pm = small_pool.tile([128, 2 * NT], F32, name="pm")
            for t, w in enumerate(WIDTHS_A):
                sq = scr_pool.tile([128, 2048], F32, name="sq", tag="sq")
                nc.scalar.activation(
                    sq[:, 0:w], vts[t][:, 0:w], AF.Square,
                    accum_out=pm[:, t : t + 1],
                )
                nc.vector.tensor_reduce(
                    pm[:, NT + t : NT + t + 1], vts[t][:, 0:w],
                    mybir.AxisListType.X, ALU.add,
                )
            nc.sync.dma_start(out_pm[:], pm[:])

    nc.compile()
    return nc


def build_kernel_b(debug=False):
    """Main kernel: factored C-build + two bf16 matmul chains.

    C^T_re[j,i] = a_i * (sp_j/h_ij) * mask,  C^T_im[j,i] = sp_i * m_j *
    (sp_j/h_ij) * mask, with h = sqrt(m2_i + m2_j).  The trick: fold sp_j
    into the Sqrt activation itself — h'_jt = sqrt(m2_i/p_j + m2_j/p_j)
    via per-partition scale/bias, so 1/h' = sp_j/h directly.  Then
      rm  = mask(1/h')        (re stationary, bf16)
      rm2 = m_j * rm          (im stationary, one cheap scale)
    and the moving operand is plain bf16 E columns.  Per-i factors (a_i,
    sp_i) fold into the PSUM->SBUF copies of T.  m2 is partition-broadcast
    on the PE from a 2KB row input (a K=1 matmul during warmup) so nothing
    waits on a slow replicated DMA.
    """
    nc = bacc.Bacc("TRN2", target_bir_lowering=False, debug=False, num_devices=NCORES)

    # host-derived per-partition vectors:
    # cols 0:4 = 1/p, 4:8 = m^2/p, 8:12 = m, 12:16 = a=m*sp, 16:20 = sp
    vecs = nc.dram_tensor("vecs", [128, 5 * KT], F32, kind="ExternalInput")
    # [1, 640]: cols 0:512 = tf32(m^2) row, 512:640 = 1.0 (bcast stationary)
    pmrow = nc.dram_tensor("pmrow", [1, D + 128], F32R, kind="ExternalInput")
    efull = nc.dram_tensor("efull", [KT, 128, S], BF16, kind="ExternalInput")
    # all 4 kt-blocks of E's column shard packed along free: 2KB DMA lines
    ecall = nc.dram_tensor("ecall", [128, KT * COLS_PER_CORE], BF16,
                           kind="ExternalInput")
    # transposed output slabs: host transposes back (out[:, cols] = slab.T)
    out_re = nc.dram_tensor("out_re", [COLS_PER_CORE, S], BF16, kind="ExternalOutput")
    out_im = nc.dram_tensor("out_im", [COLS_PER_CORE, S], BF16, kind="ExternalOutput")
    if debug:
        dbg_rm = nc.dram_tensor("dbg_rm", [KT, 128, D], BF16, kind="ExternalOutput")
        dbg_ep = nc.dram_tensor("dbg_ep", [KT, 128, COLS_PER_CORE], BF16,
                                kind="ExternalOutput")
        dbg_ea = nc.dram_tensor("dbg_ea", [KT, 128, COLS_PER_CORE], BF16,
                                kind="ExternalOutput")
        dbg_t = nc.dram_tensor("dbg_t", [KT, 128, 2 * COLS_PER_CORE], BF16,
                               kind="ExternalOutput")

    with tile.TileContext(nc) as tc:
        with (
            tc.tile_pool(name="epool", bufs=1) as e_pool,
            tc.tile_pool(name="small", bufs=1) as small_pool,
            tc.tile_pool(name="cbuild", bufs=2) as cb_pool,
            tc.tile_pool(name="ctp", bufs=1) as ct_pool,
            tc.tile_pool(name="tsb", bufs=1) as t_pool,
            tc.tile_pool(name="ost", bufs=4) as o_pool,
            tc.tile_pool(name="psA", bufs=4, space="PSUM") as psA,
            tc.tile_pool(name="psB", bufs=4, space="PSUM") as psB,
        ):
            # -------- input DMAs (pmrow + ec first, then E over queues) -------
            pmrow_sb = small_pool.tile([1, D + 128], F32R, name="pmrow_sb")
            nc.sync.dma_start(pmrow_sb[:], pmrow[:])
            vv = small_pool.tile([128, 5 * KT], F32, name="vv")
            nc.scalar.dma_start(vv[:], vecs[:])
            ec_all = e_pool.tile([128, KT * COLS_PER_CORE], BF16, name="ecall")
            nc.sync.dma_start(ec_all[:], ecall[:])
            invp4 = vv[:, 0:KT]
            m2p4 = vv[:, KT : 2 * KT]
            m4 = vv[:, 2 * KT : 3 * KT]
            a4 = vv[:, 3 * KT : 4 * KT]
            sp4 = vv[:, 4 * KT : 5 * KT]

            e_sb = []
            for kt in range(KT):
                et = e_pool.tile([128, S], BF16, name=f"e{kt}", tag=f"e{kt}")
                (nc.sync if kt % 2 == 0 else nc.scalar).dma_start(et[:], efull[kt])
                e_sb.append(et)

            # -------- PE: m2 partition-broadcast first, then warms ------------
            # the warms use the pmrow ones-slice as stationary so they DEPEND
            # on the same DMA as the broadcast — the static scheduler then
            # keeps the broadcast first instead of hoisting dep-free warms
            warm_b = small_pool.tile([128, 512], BF16, name="warm_b")
            nc.gpsimd.memset(warm_b[:], 0.001)
            ones1 = pmrow_sb[0:1, D : D + 128]
            ps_m2 = psB.tile([128, D], F32, name="ps_m2", tag="o")
            nc.tensor.matmul(ps_m2[:], ones1, pmrow_sb[0:1, 0:D],
                             start=True, stop=True)
            ps_w = psB.tile([128, 512], F32, name="ps_w", tag="o")
            for i in range(WARMUP):
                nc.tensor.matmul(
                    ps_w[:], ones1, pmrow_sb[0:1, 0:512],
                    start=(i == 0), stop=(i == WARMUP - 1),
                )

            # -------- C-build: h' = h/sp_j via scaled Sqrt, then mask ---------
            CC = COLS_PER_CORE
            rm_sb, rm2_sb = [], []
            for jt in range(KT):
                h = cb_pool.tile([128, D], F32, name="h", tag="h")
                nc.scalar.activation(
                    h[:], ps_m2[:], AF.Sqrt,
                    bias=m2p4[:, jt : jt + 1], scale=invp4[:, jt : jt + 1],
                )
                rinv = cb_pool.tile([128, D], F32, name="rinv", tag="rinv")
                nc.vector.reciprocal_approx_fast(out=rinv[:], in_=h[:])
                rm = ct_pool.tile([128, D], BF16, name=f"rm{jt}", tag=f"rm{jt}")
                nc.gpsimd.affine_select(
                    out=rm[:], in_=rinv[:],
                    pattern=[[-1, D]], compare_op=ALU.is_gt,
                    fill=0.0, base=128 * jt, channel_multiplier=1,
                )
                rm_sb.append(rm)
                rm2 = ct_pool.tile([128, D], BF16, name=f"rn{jt}", tag=f"rn{jt}")
                nc.vector.tensor_scalar(
                    rm2[:], rm[:], m4[:, jt : jt + 1], None, ALU.mult
                )
                rm2_sb.append(rm2)

            # -------- T chain: ps_t[it] = [sum_j rm*ep | sum_j rm*ea] ---------
            ps_ts = [
                psA.tile(
                    [128, 2 * COLS_PER_CORE], F32, name=f"ps_t{it}", tag=f"t{it}",
                    bufs=1,
                )
                for it in range(KT)
            ]
            # part-outer order: only ONE open accumulation group per PSUM bank
            # (interleaving re/im groups in one bank corrupts the first write)
            t_sb = [
                t_pool.tile([128, 2 * CC], BF16, name=f"tsb{it}", tag=f"tsb{it}")
                for it in range(KT)
            ]
            scale4 = (a4, sp4)
            for pi, (lo, stat) in enumerate(((0, rm_sb), (CC, rm2_sb))):
                for jt in range(KT):
                    for it in range(KT):
                        nc.tensor.matmul(
                            ps_ts[it][:, lo : lo + CC],
                            stat[jt][:, it * 128 : (it + 1) * 128],
                            ec_all[:, jt * CC : (jt + 1) * CC],
                            start=(jt == 0), stop=(jt == KT - 1),
                        )
                # this part's halves are complete: produce the scaled bf16
                # t_sb copies while the PE streams the next pass
                for it in range(KT):
                    nc.vector.tensor_scalar(
                        t_sb[it][:, lo : lo + CC], ps_ts[it][:, lo : lo + CC],
                        scale4[pi][:, it : it + 1], None, ALU.mult,
                    )
            if debug:
                for jt in range(KT):
                    nc.sync.dma_start(dbg_rm[jt], rm_sb[jt][:])
                    nc.sync.dma_start(dbg_ep[jt], rm2_sb[jt][:, 0:COLS_PER_CORE])
                    nc.sync.dma_start(dbg_t[jt], t_sb[jt][:])
            # bridge the copy latency so the PE never idles (an idle gap
            # triggers a ~7us half-rate HAM window); stationary depends on
            # t_sb[0] so the scheduler cannot hoist these earlier
            for i in range(POST_FILL):
                nc.tensor.matmul(
                    ps_w[:], t_sb[0][:, 0:128], warm_b[:],
                    start=(i == 0), stop=(i == POST_FILL - 1),
                )

            # -------- out^T[cols, :] = T^T @ E  (transposed chain) ------------
            NS = S // 512
            cnt = 0
            for part, outT in ((0, out_re), (1, out_im)):
                for mc in range(2):
                    b = part * 2 + mc
                    c0 = part * CC + mc * 128
                    if b % 2 == 0:
                        pso = [
                            psB.tile([128, 512], F32, name=f"pso{sn}", tag="o")
                            for sn in range(NS)
                        ]
                    else:
                        # alternate PSUM pools so this block's matmuls don't
                        # wait on the previous block's PSUM->SBUF copies
                        pso = [
                            psA.tile([128, 512], F32, name=f"psoA{sn}",
                                     tag=f"t{sn}", bufs=1)
                            for sn in range(NS)
                        ]
                    if b < 3:
                        for it in range(KT):
                            for sn in range(NS):
                                nc.tensor.matmul(
                                    pso[sn][:],
                                    t_sb[it][:, c0 : c0 + 128],
                                    e_sb[it][:, sn * 512 : (sn + 1) * 512],
                                    start=(it == 0), stop=(it == KT - 1),
                                )
                    else:
                        # final block runs sn-outer: each pso completes early
                        # so its copy+DMA pipeline inside the block instead of
                        # serializing after the last matmul (shorter tail)
                        for sn in range(NS):
                            for it in range(KT):
                                nc.tensor.matmul(
                                    pso[sn][:],
                                    t_sb[it][:, c0 : c0 + 128],
                                    e_sb[it][:, sn * 512 : (sn + 1) * 512],
                                    start=(it == 0), stop=(it == KT - 1),
                                )
                    # pair two 512-chunks per write DMA: 2KB DRAM lines
                    for sn2 in range(NS // 2):
                        osb = o_pool.tile([128, 1024], BF16, name="osb", tag="osb")
                        nc.vector.tensor_copy(osb[:, 0:512], pso[2 * sn2][:])
                        nc.scalar.copy(osb[:, 512:1024], pso[2 * sn2 + 1][:])
                        eng = nc.sync if cnt % 2 == 0 else nc.scalar
                        eng.dma_start(
                            outT[mc * 128 : (mc + 1) * 128,
                                 sn2 * 1024 : (sn2 + 1) * 1024],
                            osb[:],
                        )
                        cnt += 1

    nc.compile()
    return nc


def _prepare_a_in_maps(vulns):
    vulns = np.ascontiguousarray(np.asarray(vulns, dtype=np.float32))
    in_maps = []
    for c in range(NCORES):
        vsh = vulns[c * ROWS_PER_CORE : (c + 1) * ROWS_PER_CORE]
        in_maps.append(
            {"v128": np.ascontiguousarray(vsh.reshape(128, -1))}
        )
    return in_maps


def _reduce_a(res_a):
    """Finish the p/msum reduction from the per-tile partials (host, tiny)."""
    p_full = np.empty(D, dtype=np.float32)
    msum_full = np.empty(D, dtype=np.float32)
    for c in range(NCORES):
        pm = res_a.results[c]["out_pm"].astype(np.float64)
        p128 = pm[:, 0:NT].sum(axis=1)
        m128 = pm[:, NT : 2 * NT].sum(axis=1)
        sl = slice(c * ROWS_PER_CORE, (c + 1) * ROWS_PER_CORE)
        p_full[sl] = p128.reshape(-1, 2).sum(axis=1)
        msum_full[sl] = m128.reshape(-1, 2).sum(axis=1)
    return p_full, msum_full


def _prepare_b_in_maps(embed_table, domain_ids, p_full, msum_full):
    embed_table = np.ascontiguousarray(np.asarray(embed_table, dtype=np.float32))
    domain_ids = np.asarray(domain_ids).astype(np.int64)
    E = np.ascontiguousarray(embed_table[domain_ids])  # [512, 2048]
    e_bf = E.astype(ml_dtypes.bfloat16).reshape(KT, 128, S)
    # tiny derived vectors (the [512]-sized sharding prep)
    p64 = p_full.astype(np.float64)
    m64 = msum_full.astype(np.float64) * INV_V
    sp = np.sqrt(p64).astype(np.float32)
    m = m64.astype(np.float32)
    a = (m64 * np.sqrt(p64)).astype(np.float32)
    m2 = (m64 * m64).astype(np.float32)
    invp = (1.0 / p64).astype(np.float32)
    m2p = (m64 * m64 / p64).astype(np.float32)

    def pp(x):
        return x.reshape(KT, 128).T

    vecs = np.empty((128, 5 * KT), dtype=np.float32)
    vecs[:, 0:KT] = pp(invp)
    vecs[:, KT : 2 * KT] = pp(m2p)
    vecs[:, 2 * KT : 3 * KT] = pp(m)
    vecs[:, 3 * KT : 4 * KT] = pp(a)
    vecs[:, 4 * KT : 5 * KT] = pp(sp)
    pmrow = np.empty((1, D + 128), dtype=np.float32)
    pmrow[0, 0:D] = _tf32_round(m2)
    pmrow[0, D:] = 1.0
    in_maps = []
    for c in range(NCORES):
        csl = slice(c * COLS_PER_CORE, (c + 1) * COLS_PER_CORE)
        ecall = np.ascontiguousarray(
            np.concatenate([e_bf[kt, :, csl] for kt in range(KT)], axis=1)
        )
        in_maps.append(
            {
                "vecs": vecs,
                "pmrow": pmrow,
                "efull": e_bf,
                "ecall": ecall,
            }
        )
    return in_maps


def kernel(vulns, embed_table, domain_ids, _trace=False):
    if "nc_a" not in _CACHE:
        _CACHE["nc_a"] = build_kernel_a()
    if "nc_b" not in _CACHE:
        _CACHE["nc_b"] = build_kernel_b()

    res_a = run_bass_kernel_spmd(
        _CACHE["nc_a"], _prepare_a_in_maps(vulns),
        core_ids=list(range(NCORES)), trace=_trace,
    )
    _CACHE["res_a"] = res_a
    p_full, msum_full = _reduce_a(res_a)

    res_b = run_bass_kernel_spmd(
        _CACHE["nc_b"], _prepare_b_in_maps(embed_table, domain_ids, p_full, msum_full),
        core_ids=list(range(NCORES)), trace=_trace,
    )
    _CACHE["res_b"] = res_b

    out = np.empty((S, S), dtype=np.complex64)
    for c in range(NCORES):
        r = res_b.results[c]
        sl = slice(c * COLS_PER_CORE, (c + 1) * COLS_PER_CORE)
        out[:, sl] = (
            r["out_re"].astype(np.float32).T
            + 1j * r["out_im"].astype(np.float32).T
        )
    return out


if __name__ == "__main__":
    rng = np.random.default_rng(0)
    v = rng.standard_normal((D, V), dtype=np.float32)
    et = rng.standard_normal((D, S), dtype=np.float32)
    ids = np.arange(D, dtype=np.int32)
    out = kernel(v, et, ids)
    print(out.shape, out.dtype)


# revision 70
# speedup vs baseline: 1.1449x; 1.1449x over previous
"""Trainium2 Bass kernel for nn_OmegaEntangle (E^T C E with entangle coefficients).

Math (validated vs reference):
  p_i = sum_j v_ij^2 ; m_i = mean_j v_ij
  C[i,j] = mask(i<j) * sqrt(p_i p_j) * (m_i + 1j*m_j) / sqrt(m_i^2 + m_j^2)
  out = E^T C E   (complex, E real)  ->  out_re = E^T Cr E, out_im = E^T Ci E

Sharding: data-parallel over the 2048 OUTPUT COLUMNS (256 per core), with the
p/m reduction row-sharded (64 rows per core).

Two NEFF launches (a device collective costs far more than host concat of the
tiny reduction result on this platform):
  Kernel A: each core reduces its [64, 32768] vuln shard into per-tile
    partial sums [128, 2*NT]; the host finishes the (tiny) reduction.
  Kernel B: each core broadcasts p/m across partitions via a K=1 matmul,
    derives the coefficient vectors, builds C^T in bf16, computes
    T = C @ E[:, cols] and out[:, cols] = E^T @ T in bf16 matmuls
    (fp32 PSUM), writes bf16 [256, 2048] slabs.
Host concatenates slabs along columns -> [2048, 2048] complex64.

Perf notes:
  - TRN2 has two HWDGE queues (SP + Activation), ~190 GB/s each; every
    large DMA set is split across both to reach the ~358 GB/s core limit.
  - bf16 matmul streams 1 row/cycle (same as f32r) at half the DMA bytes.
  - PE p-state ramps to full clock only after ~3us of continuous busy;
    warmup matmuls keep it streaming until the C tiles are ready.
"""

import numpy as np
import ml_dtypes

import concourse.bass as bass
import concourse.mybir as mybir
import concourse.tile as tile
from concourse import bacc
from concourse.bass_utils import run_bass_kernel_spmd

D = 512          # number of domains
V = 32768        # vuln dim
S = 2048         # sup (embed) dim
NCORES = 8
ROWS_PER_CORE = D // NCORES          # 64
COLS_PER_CORE = S // NCORES          # 256
KT = D // 128                         # 4 contraction tiles
INV_V = 1.0 / V

# kernel A tiling: narrower tiles at the end shorten the compute tail
WIDTHS_A = [2048] * 6 + [1024] * 4
NT = len(WIDTHS_A)

WARMUP = 6                            # PE warm matmuls until C tiles are ready
POST_FILL = 6                         # fillers covering the T->out copy gap

F32 = mybir.dt.float32
F32R = mybir.dt.float32r
BF16 = mybir.dt.bfloat16
AF = mybir.ActivationFunctionType
ALU = mybir.AluOpType

_CACHE = {}


def _tf32_round(x):
    xi = np.ascontiguousarray(x, dtype=np.float32).view(np.uint32)
    return ((xi + np.uint32(0x1000)) & np.uint32(0xFFFFE000)).view(np.float32)


def build_kernel_a():
    """Reduce kernel: per-tile partial p/msum over the 64-row vuln shard.

    DMA-bound: 8 MiB/core of vulns split across both HWDGE queues.
    Scalar does Square+accum, vector does row-sum reduce; host finishes.
    """
    nc = bacc.Bacc("TRN2", target_bir_lowering=False, debug=False, num_devices=NCORES)

    v128 = nc.dram_tensor("v128", [128, V * ROWS_PER_CORE // 128], F32,
                          kind="ExternalInput")
    out_pm = nc.dram_tensor("out_pm", [128, 2 * NT], F32, kind="ExternalOutput")

    with tile.TileContext(nc) as tc:
        with (
            tc.tile_pool(name="vin", bufs=5) as vin_pool,
            tc.tile_pool(name="scr", bufs=2) as scr_pool,
            tc.tile_pool(name="small", bufs=1) as small_pool,
        ):
            # all DMA triggers first, spread over 3 queues (2 HWDGE + SWDGE).
            # scalar goes last in the rotation: its ACT_TABLE_LOAD (for the
            # Square activation) delays its first trigger by ~1.3us.
            vts = []
            off = 0
            # arrival order must match program order: sync starts ~8.7us,
            # scalar ~10 (after its ACT table), SWDGE ~12 — so t0/t1/t2 land
            # in sequence and the per-tile compute pipeline never stalls
            engs = [nc.sync, nc.scalar, nc.gpsimd]
            for t, w in enumerate(WIDTHS_A):
                vt = vin_pool.tile([128, 2048], F32, name=f"vt{t}", tag="vt")
                engs[t % 3].dma_start(vt[:, 0:w], v128[:, off : off + w])
                off += w
                vts.append(vt)

            # squares on scalar, row-sums on vector (each ~18-20us of engine
            # time; one engine cannot absorb both)
            pm = small_pool.tile([128, 2 * NT], F32, name="pm")
            for t, w in enumerate(WIDTHS_A):
                sq = scr_pool.tile([128, 2048], F32, name="sq", tag="sq")
                nc.scalar.activation(
                    sq[:, 0:w], vts[t][:, 0:w], AF.Square,
                    accum_out=pm[:, t : t + 1],
                )
                nc.vector.tensor_reduce(
                    pm[:, NT + t : NT + t + 1], vts[t][:, 0:w],
                    mybir.AxisListType.X, ALU.add,
                )
            nc.sync.dma_start(out_pm[:], pm[:])

    nc.compile()
    return nc


def build_kernel_b(debug=False):
    """Main kernel: factored C-build + two bf16 matmul chains.

    C^T_re[j,i] = a_i * (sp_j/h_ij) * mask,  C^T_im[j,i] = sp_i * m_j *
    (sp_j/h_ij) * mask, with h = sqrt(m2_i + m2_j).  The trick: fold sp_j
    into the Sqrt activation itself — h'_jt = sqrt(m2_i/p_j + m2_j/p_j)
    via per-partition scale/bias, so 1/h' = sp_j/h directly.  Then
      rm  = mask(1/h')        (re stationary, bf16)
      rm2 = m_j * rm          (im stationary, one cheap scale)
    and the moving operand is plain bf16 E columns.  Per-i factors (a_i,
    sp_i) fold into the PSUM->SBUF copies of T.  m2 is partition-broadcast
    on the PE from a 2KB row input (a K=1 matmul during warmup) so nothing
    waits on a slow replicated DMA.
    """
    nc = bacc.Bacc("TRN2", target_bir_lowering=False, debug=False, num_devices=NCORES)

    # host-derived per-partition vectors:
    # cols 0:4 = 1/p, 4:8 = m^2/p, 8:12 = m, 12:16 = a=m*sp, 16:20 = sp
    vecs = nc.dram_tensor("vecs", [128, 5 * KT], F32, kind="ExternalInput")
    # [1, 640]: cols 0:512 = tf32(m^2) row, 512:640 = 1.0 (bcast stationary)
    pmrow = nc.dram_tensor("pmrow", [1, D + 128], F32R, kind="ExternalInput")
    efull = nc.dram_tensor("efull", [KT, 128, S], BF16, kind="ExternalInput")
    # all 4 kt-blocks of E's column shard packed along free: 2KB DMA lines
    ecall = nc.dram_tensor("ecall", [128, KT * COLS_PER_CORE], BF16,
                           kind="ExternalInput")
    # transposed output slabs: host transposes back (out[:, cols] = slab.T)
    out_re = nc.dram_tensor("out_re", [COLS_PER_CORE, S], BF16, kind="ExternalOutput")
    out_im = nc.dram_tensor("out_im", [COLS_PER_CORE, S], BF16, kind="ExternalOutput")
    if debug:
        dbg_rm = nc.dram_tensor("dbg_rm", [KT, 128, D], BF16, kind="ExternalOutput")
        dbg_ep = nc.dram_tensor("dbg_ep", [KT, 128, COLS_PER_CORE], BF16,
                                kind="ExternalOutput")
        dbg_ea = nc.dram_tensor("dbg_ea", [KT, 128, COLS_PER_CORE], BF16,
                                kind="ExternalOutput")
        dbg_t = nc.dram_tensor("dbg_t", [KT, 128, 2 * COLS_PER_CORE], BF16,
                               kind="ExternalOutput")

    with tile.TileContext(nc) as tc:
        with (
            tc.tile_pool(name="epool", bufs=1) as e_pool,
            tc.tile_pool(name="small", bufs=1) as small_pool,
            tc.tile_pool(name="cbuild", bufs=2) as cb_pool,
            tc.tile_pool(name="ctp", bufs=1) as ct_pool,
            tc.tile_pool(name="tsb", bufs=1) as t_pool,
            tc.tile_pool(name="ost", bufs=4) as o_pool,
            tc.tile_pool(name="psA", bufs=4, space="PSUM") as psA,
            tc.tile_pool(name="psB", bufs=4, space="PSUM") as psB,
        ):
            # -------- input DMAs (pmrow + ec first, then E over queues) -------
            pmrow_sb = small_pool.tile([1, D + 128], F32R, name="pmrow_sb")
            nc.sync.dma_start(pmrow_sb[:], pmrow[:])
            vv = small_pool.tile([128, 5 * KT], F32, name="vv")
            nc.scalar.dma_start(vv[:], vecs[:])
            ec_all = e_pool.tile([128, KT * COLS_PER_CORE], BF16, name="ecall")
            nc.sync.dma_start(ec_all[:], ecall[:])
            invp4 = vv[:, 0:KT]
            m2p4 = vv[:, KT : 2 * KT]
            m4 = vv[:, 2 * KT : 3 * KT]
            a4 = vv[:, 3 * KT : 4 * KT]
            sp4 = vv[:, 4 * KT : 5 * KT]

            e_sb = []
            for kt in range(KT):
                et = e_pool.tile([128, S], BF16, name=f"e{kt}", tag=f"e{kt}")
                (nc.sync if kt % 2 == 0 else nc.scalar).dma_start(et[:], efull[kt])
                e_sb.append(et)

            # -------- PE: m2 partition-broadcast first, then warms ------------
            # the warms use the pmrow ones-slice as stationary so they DEPEND
            # on the same DMA as the broadcast — the static scheduler then
            # keeps the broadcast first instead of hoisting dep-free warms
            warm_b = small_pool.tile([128, 512], BF16, name="warm_b")
            nc.gpsimd.memset(warm_b[:], 0.001)
            ones1 = pmrow_sb[0:1, D : D + 128]
            ps_m2 = psB.tile([128, D], F32, name="ps_m2", tag="o")
            nc.tensor.matmul(ps_m2[:], ones1, pmrow_sb[0:1, 0:D],
                             start=True, stop=True)
            ps_w = psB.tile([128, 512], F32, name="ps_w", tag="o")
            for i in range(WARMUP):
                nc.tensor.matmul(
                    ps_w[:], ones1, pmrow_sb[0:1, 0:512],
                    start=(i == 0), stop=(i == WARMUP - 1),
                )

            # -------- C-build: h' = h/sp_j via scaled Sqrt, then mask ---------
            CC = COLS_PER_CORE
            rm_sb, rm2_sb = [], []
            for jt in range(KT):
                h = cb_pool.tile([128, D], F32, name="h", tag="h")
                nc.scalar.activation(
                    h[:], ps_m2[:], AF.Sqrt,
                    bias=m2p4[:, jt : jt + 1], scale=invp4[:, jt : jt + 1],
                )
                rinv = cb_pool.tile([128, D], F32, name="rinv", tag="rinv")
                nc.vector.reciprocal_approx_fast(out=rinv[:], in_=h[:])
                rm = ct_pool.tile([128, D], BF16, name=f"rm{jt}", tag=f"rm{jt}")
                nc.gpsimd.affine_select(
                    out=rm[:], in_=rinv[:],
                    pattern=[[-1, D]], compare_op=ALU.is_gt,
                    fill=0.0, base=128 * jt, channel_multiplier=1,
                )
                rm_sb.append(rm)
                rm2 = ct_pool.tile([128, D], BF16, name=f"rn{jt}", tag=f"rn{jt}")
                nc.vector.tensor_scalar(
                    rm2[:], rm[:], m4[:, jt : jt + 1], None, ALU.mult
                )
                rm2_sb.append(rm2)

            # -------- T chain: ps_t[it] = [sum_j rm*ep | sum_j rm*ea] ---------
            ps_ts = [
                psA.tile(
                    [128, 2 * COLS_PER_CORE], F32, name=f"ps_t{it}", tag=f"t{it}",
                    bufs=1,
                )
                for it in range(KT)
            ]
            # part-outer order: only ONE open accumulation group per PSUM bank
            # (interleaving re/im groups in one bank corrupts the first write)
            t_sb = [
                t_pool.tile([128, 2 * CC], BF16, name=f"tsb{it}", tag=f"tsb{it}")
                for it in range(KT)
            ]
            scale4 = (a4, sp4)
            for pi, (lo, stat) in enumerate(((0, rm_sb), (CC, rm2_sb))):
                for jt in range(KT):
                    for it in range(KT):
                        nc.tensor.matmul(
                            ps_ts[it][:, lo : lo + CC],
                            stat[jt][:, it * 128 : (it + 1) * 128],
                            ec_all[:, jt * CC : (jt + 1) * CC],
                            start=(jt == 0), stop=(jt == KT - 1),
                        )
                # this part's halves are complete: produce the scaled bf16
                # t_sb copies while the PE streams the next pass
                for it in range(KT):
                    nc.vector.tensor_scalar(
                        t_sb[it][:, lo : lo + CC], ps_ts[it][:, lo : lo + CC],
                        scale4[pi][:, it : it + 1], None, ALU.mult,
                    )
            if debug:
                for jt in range(KT):
                    nc.sync.dma_start(dbg_rm[jt], rm_sb[jt][:])
                    nc.sync.dma_start(dbg_ep[jt], rm2_sb[jt][:, 0:COLS_PER_CORE])
                    nc.sync.dma_start(dbg_t[jt], t_sb[jt][:])
            # bridge the copy latency so the PE never idles (an idle gap
            # triggers a ~7us half-rate HAM window); stationary depends on
            # t_sb[0] so the scheduler cannot hoist these earlier
            for i in range(POST_FILL):
                nc.tensor.matmul(
                    ps_w[:], t_sb[0][:, 0:128], warm_b[:],
                    start=(i == 0), stop=(i == POST_FILL - 1),
                )

            # -------- out^T[cols, :] = T^T @ E  (transposed chain) ------------
            NS = S // 512
            cnt = 0
            for part, outT in ((0, out_re), (1, out_im)):
                for mc in range(2):
                    b = part * 2 + mc
                    c0 = part * CC + mc * 128
                    if b % 2 == 0:
                        pso = [
                            psB.tile([128, 512], F32, name=f"pso{sn}", tag="o")
                            for sn in range(NS)
                        ]
                    else:
                        # alternate PSUM pools so this block's matmuls don't
                        # wait on the previous block's PSUM->SBUF copies
                        pso = [
                            psA.tile([128, 512], F32, name=f"psoA{sn}",
                                     tag=f"t{sn}", bufs=1)
                            for sn in range(NS)
                        ]
                    # sn-outer: each pso completes a quarter into the block,
                    # so its copy+DMA pipeline inside the block (shorter tail,
                    # earlier PSUM release for the next block's allocation)
                    for sn in range(NS):
                        for it in range(KT):
                            nc.tensor.matmul(
                                pso[sn][:],
                                t_sb[it][:, c0 : c0 + 128],
                                e_sb[it][:, sn * 512 : (sn + 1) * 512],
                                start=(it == 0), stop=(it == KT - 1),
                            )
                    # pair two 512-chunks per write DMA: 2KB DRAM lines
                    for sn2 in range(NS // 2):
                        osb = o_pool.tile([128, 1024], BF16, name="osb", tag="osb")
                        nc.vector.tensor_copy(osb[:, 0:512], pso[2 * sn2][:])
                        nc.scalar.copy(osb[:, 512:1024], pso[2 * sn2 + 1][:])
                        eng = nc.sync if cnt % 2 == 0 else nc.scalar
                        eng.dma_start(
                            outT[mc * 128 : (mc + 1) * 128,
                                 sn2 * 1024 : (sn2 + 1) * 1024],
                            osb[:],
                        )
                        cnt += 1

    nc.compile()
    return nc


def _prepare_a_in_maps(vulns):
    vulns = np.ascontiguousarray(np.asarray(vulns, dtype=np.float32))
    in_maps = []
    for c in range(NCORES):
        vsh = vulns[c * ROWS_PER_CORE : (c + 1) * ROWS_PER_CORE]
        in_maps.append(
            {"v128": np.ascontiguousarray(vsh.reshape(128, -1))}
        )
    return in_maps


def _reduce_a(res_a):
    """Finish the p/msum reduction from the per-tile partials (host, tiny)."""
    p_full = np.empty(D, dtype=np.float32)
    msum_full = np.empty(D, dtype=np.float32)
    for c in range(NCORES):
        pm = res_a.results[c]["out_pm"].astype(np.float64)
        p128 = pm[:, 0:NT].sum(axis=1)
        m128 = pm[:, NT : 2 * NT].sum(axis=1)
        sl = slice(c * ROWS_PER_CORE, (c + 1) * ROWS_PER_CORE)
        p_full[sl] = p128.reshape(-1, 2).sum(axis=1)
        msum_full[sl] = m128.reshape(-1, 2).sum(axis=1)
    return p_full, msum_full


def _prepare_b_in_maps(embed_table, domain_ids, p_full, msum_full):
    embed_table = np.ascontiguousarray(np.asarray(embed_table, dtype=np.float32))
    domain_ids = np.asarray(domain_ids).astype(np.int64)
    E = np.ascontiguousarray(embed_table[domain_ids])  # [512, 2048]
    e_bf = E.astype(ml_dtypes.bfloat16).reshape(KT, 128, S)
    # tiny derived vectors (the [512]-sized sharding prep)
    p64 = p_full.astype(np.float64)
    m64 = msum_full.astype(np.float64) * INV_V
    sp = np.sqrt(p64).astype(np.float32)
    m = m64.astype(np.float32)
    a = (m64 * np.sqrt(p64)).astype(np.float32)
    m2 = (m64 * m64).astype(np.float32)
    invp = (1.0 / p64).astype(np.float32)
    m2p = (m64 * m64 / p64).astype(np.float32)

    def pp(x):
        return x.reshape(KT, 128).T

    vecs = np.empty((128, 5 * KT), dtype=np.float32)
    vecs[:, 0:KT] = pp(invp)
    vecs[:, KT : 2 * KT] = pp(m2p)
    vecs[:, 2 * KT : 3 * KT] = pp(m)
    vecs[:, 3 * KT : 4 * KT] = pp(a)
    vecs[:, 4 * KT : 5 * KT] = pp(sp)
    pmrow = np.empty((1, D + 128), dtype=np.float32)
    pmrow[0, 0:D] = _tf32_round(m2)
    pmrow[0, D:] = 1.0
    in_maps = []
    for c in range(NCORES):
        csl = slice(c * COLS_PER_CORE, (c + 1) * COLS_PER_CORE)
        ecall = np.ascontiguousarray(
            np.concatenate([e_bf[kt, :, csl] for kt in range(KT)], axis=1)
        )
        in_maps.append(
            {
                "vecs": vecs,
                "pmrow": pmrow,
                "efull": e_bf,
                "ecall": ecall,
            }
        )
    return in_maps


def kernel(vulns, embed_table, domain_ids, _trace=False):
    if "nc_a" not in _CACHE:
        _CACHE["nc_a"] = build_kernel_a()
    if "nc_b" not in _CACHE:
        _CACHE["nc_b"] = build_kernel_b()

    res_a = run_bass_kernel_spmd(
        _CACHE["nc_a"], _prepare_a_in_maps(vulns),
        core_ids=list(range(NCORES)), trace=_trace,
    )
    _CACHE["res_a"] = res_a
    p_full, msum_full = _reduce_a(res_a)

    res_b = run_bass_kernel_spmd(
        _CACHE["nc_b"], _prepare_b_in_maps(embed_table, domain_ids, p_full, msum_full),
        core_ids=list(range(NCORES)), trace=_trace,
    )
    _CACHE["res_b"] = res_b

    out = np.empty((S, S), dtype=np.complex64)
    for c in range(NCORES):
        r = res_b.results[c]
        sl = slice(c * COLS_PER_CORE, (c + 1) * COLS_PER_CORE)
        out[:, sl] = (
            r["out_re"].astype(np.float32).T
            + 1j * r["out_im"].astype(np.float32).T
        )
    return out


if __name__ == "__main__":
    rng = np.random.default_rng(0)
    v = rng.standard_normal((D, V), dtype=np.float32)
    et = rng.standard_normal((D, S), dtype=np.float32)
    ids = np.arange(D, dtype=np.int32)
    out = kernel(v, et, ids)
    print(out.shape, out.dtype)
